# revision 25
# baseline (speedup 1.0000x reference)
"""CRF layer gradient kernel for 8 TRN2 NeuronCores (v3).

Strategy: data-parallel over the N=2048 words axis (256 words/core, as
4 chains x 64 words packed into the 128 partitions = 4 chains x 32 labels).
Free-dim packing is position-major within a chain: column f = 64*i + w
(i = position, w = word), so every scan step reads/writes one contiguous
[128, 64] slice.

The forward-backward DP runs in the exp domain (scaled by CHAT): with
ETs = exp(T)/CHAT,
  AE[0] = E[0];   AE[i] = (AE[i-1] @ ETs)  * E[i]
  BE[63] = E[63]; BE[i] = (BE[i+1] @ ETs.T) * E[i]      (unnormalized)
  p1[i] = AE[i] * BE[i] * exp(-dots[i]) * (1/z)
Forward and backward scans are independent and chase the DMA stream from
opposite ends (dt slabs arrive 0,7,1,6,...); z is computed at the meeting
point (z = sum_k AE[t]*BE[t]*Einv[t] at any t — exact identity), so the
normalizer is ready as soon as the scans cross.  The p1 -> transpose ->
dw pipeline then runs blockwise, middle-out, overlapping the scan tails:
DVE does the p1 products while PE does transposes + dw matmuls.

PE efficiency: the K=32-wide matmuls (emission scores and dw = p1.T @ x)
are 4-way column-tiled via tile_position=(0,32c), running the four chains
concurrently in separate 32-column groups of the 128x128 array.  p1 is
transposed with 32 full-width [128,128] PE transposes.  dn and p1 travel
as fp8e4; everything else bf16 (validated ~8e-3 vs the fp32 reference).

Device returns dwn = p1.T @ x (per-chain bands) and the AE/BE marginal
factors; host computes onehot.T @ x in full precision (BLAS), subtracts,
and folds the tiny dT matrix in float64.
"""

import sys

import numpy as np

sys.path.insert(0, "/opt/trn_rl_repo")

import concourse.bass as bass
import concourse.tile as tile
from concourse import bacc, mybir
from concourse.bass_utils import run_bass_kernel_spmd

N, M, K, D = 2048, 64, 32, 512
NC = 8
WPC = N // NC          # 256 words per core
RPC = WPC * M          # 16384 rows per core
CHAT = 60.0
F32 = mybir.dt.float32
BF16 = mybir.dt.bfloat16
F8 = mybir.dt.float8e4

_CACHE = {}

# dots-block arrival order: alternate ends so fwd (blocks 0,1,..) and bwd
# (blocks 7,6,..) both chase the stream; block 4 lands last.
SLAB_ORDER = [0, 7, 1, 6, 2, 5, 3, 4]
# p1/transpose/dw block order: middle-out, matching scan completion.
BLOCK_ORDER = [3, 4, 2, 5, 1, 6, 0, 7]


def _build_module():
    nc = bacc.Bacc("TRN2", target_bir_lowering=False, debug=False)

    dt_d = nc.dram_tensor("dt", [128, 8, 16, 512], BF16, kind="ExternalInput")
    dn_d = nc.dram_tensor("dn", [128, 128, 512], F8, kind="ExternalInput")
    # merged constants, one DMA: wt(128) | etf(128) | etb(128) | id(128)
    # | oz(4) | ob(128, rows 0-3)
    cm_d = nc.dram_tensor("cm", [128, 644], BF16, kind="ExternalInput")
    dw_d = nc.dram_tensor("dw", [128, 512], F32, kind="ExternalOutput")
    ae_d = nc.dram_tensor("ae", [128, 4096], BF16, kind="ExternalOutput")
    be_d = nc.dram_tensor("be", [128, 4096], BF16, kind="ExternalOutput")

    with tile.TileContext(nc) as tc:
        _kernel_body(tc, nc, dt_d, dn_d, cm_d, dw_d, ae_d, be_d)
    nc.compile()
    return nc


def _kernel_body(tc, nc, dt_d, dn_d, cm_d, dw_d, ae_d, be_d):
    from contextlib import ExitStack
    Act = mybir.ActivationFunctionType
    ctx = ExitStack()
    with ctx:
        consts = ctx.enter_context(tc.tile_pool(name="consts", bufs=1))
        big = ctx.enter_context(tc.tile_pool(name="big", bufs=1))
        dtp = ctx.enter_context(tc.tile_pool(name="dtp", bufs=3))

        cm_t = consts.tile([128, 644], BF16)

        def wtc(g):
            return cm_t[:, 32 * g:32 * g + 32]
        etf_ap = cm_t[:, 128:256]
        etb_ap = cm_t[:, 256:384]
        id_ap = cm_t[:, 384:512]
        oz_ap = cm_t[:, 512:516]
        ob_ap = cm_t[0:4, 516:644]

        e_t = big.tile([128, 4096], BF16, tag="e")
        einv_t = big.tile([128, 4096], BF16, tag="einv")
        ae_t = big.tile([128, 4096], BF16, tag="ae")
        be_t = big.tile([128, 4096], BF16, tag="be")
        p1_t = big.tile([128, 4096], BF16, tag="p1")
        dn_t = big.tile([128, 128, 512], F8, tag="dn")
        rzb_t = consts.tile([128, 64], BF16)

        def blk(t):            # contiguous 64-col slice for position t
            return slice(64 * t, 64 * t + 64)

        scn = ctx.enter_context(tc.tile_pool(name="scn", bufs=3, space="PSUM"))
        trp = ctx.enter_context(tc.tile_pool(name="trp", bufs=2, space="PSUM"))
        dwp = ctx.enter_context(tc.tile_pool(name="dwp", bufs=1, space="PSUM"))
        tgp = ctx.enter_context(tc.tile_pool(name="tgp", bufs=3))

        dw_ps = dwp.tile([128, 512], F32)
        st = {"fwd": 1, "bwd": 62, "fbanks": 0, "bbanks": 0,
              "z_done": False, "blocks": list(BLOCK_ORDER), "nj": 0,
              "exported": False}

        def emit_fwd(t):
            aps = scn.tile([128, 64], F32, tag="s")
            nc.tensor.matmul(aps[:], etf_ap, ae_t[:, blk(t - 1)],
                             start=True, stop=True)
            nc.vector.tensor_mul(ae_t[:, blk(t)], aps[:], e_t[:, blk(t)])

        def emit_bwd(u):
            bps = scn.tile([128, 64], F32, tag="s")
            nc.tensor.matmul(bps[:], etb_ap, be_t[:, blk(u + 1)],
                             start=True, stop=True)
            nc.vector.tensor_mul(be_t[:, blk(u)], bps[:], e_t[:, blk(u)])

        def emit_z():
            # z = sum_k AE[32]*BE[32]*Einv[32] (exact at any position);
            # broadcast 1/z to all partitions via ones-block matmuls.
            tmp = consts.tile([128, 64], BF16)
            nc.vector.tensor_mul(tmp[:], ae_t[:, blk(32)], be_t[:, blk(32)])
            nc.vector.tensor_mul(tmp[:], tmp[:], einv_t[:, blk(32)])
            z_ps = scn.tile([4, 64], F32, tag="s")
            nc.tensor.matmul(z_ps[:], oz_ap, tmp[:], start=True, stop=True)
            rz_s = consts.tile([4, 64], BF16)
            with nc.allow_low_precision(reason="rz bf16 validated to 8e-3"):
                nc.vector.reciprocal(rz_s[:], z_ps[:])
            rzb_ps = scn.tile([128, 64], F32, tag="s")
            nc.tensor.matmul(rzb_ps[:], ob_ap, rz_s[:], start=True,
                             stop=True)
            nc.vector.tensor_copy(rzb_t[:], rzb_ps[:])

        def emit_block(b):
            # p1 = (AE*BE) * (Einv*rz) on this 512-col block; the two
            # products run on the otherwise-idle GpSimd engine so the DVE
            # (scan-critical) only pays one multiply per block.  Then
            # transpose the four 128-col strips and feed the column-tiled
            # dw matmul.
            L = slice(512 * b, 512 * b + 512)
            nc.gpsimd.tensor_mul(p1_t[:, L], ae_t[:, L], be_t[:, L])
            e3 = einv_t[:, L].rearrange("p (i w) -> p i w", i=8)
            rb = rzb_t[:].unsqueeze(1).broadcast_to([128, 8, 64])
            nc.gpsimd.tensor_mul(e3, e3, rb)
            nc.vector.tensor_mul(p1_t[:, L], p1_t[:, L], einv_t[:, L])
            for jj in range(4):
                j = 4 * b + jj
                tr = trp.tile([128, 128], BF16, tag="tr")
                nc.tensor.transpose(tr[:], p1_t[:, 128 * j:128 * j + 128],
                                    id_ap)
                tg = tgp.tile([128, 128], F8, tag="tg")
                nc.scalar.activation(tg[:], tr[:], Act.Copy)
                for c in range(4):
                    nc.tensor.matmul(
                        dw_ps[32 * c:32 * c + 32, :],
                        tg[:, 32 * c:32 * c + 32], dn_t[:, 4 * j + c, :],
                        start=(st["nj"] == 0), stop=(st["nj"] == 31),
                        tile_position=(0, 32 * c))
                st["nj"] += 1

        def pump():
            # emit everything whose inputs are covered: scan steps, then z
            # at the crossing, then completed p1/dw blocks (middle-out).
            while True:
                can_f = st["fwd"] <= 63 and (st["fwd"] // 8) < st["fbanks"]
                can_b = st["bwd"] >= 0 and (st["bwd"] // 8) >= 8 - st["bbanks"]
                if not (can_f or can_b):
                    break
                if can_f:
                    emit_fwd(st["fwd"])
                    st["fwd"] += 1
                if can_b:
                    emit_bwd(st["bwd"])
                    st["bwd"] -= 1
                if not st["z_done"] and st["fwd"] > 32 and st["bwd"] < 32:
                    emit_z()
                    st["z_done"] = True
                while st["blocks"]:
                    b = st["blocks"][0]
                    if st["fwd"] > 8 * b + 7 and st["bwd"] < 8 * b \
                            and st["z_done"]:
                        emit_block(st["blocks"].pop(0))
                    else:
                        break
            if st["fwd"] > 63 and st["bwd"] < 0 and not st["exported"]:
                # marginal factors out as soon as the scans finish, on the
                # scalar DMA queue so the final dw export (sync queue)
                # doesn't wait behind these 2 MB.
                nc.scalar.dma_start(ae_d.ap(), ae_t[:])
                nc.scalar.dma_start(be_d.ap(), be_t[:])
                st["exported"] = True

        # ---- emission-score stream + scans + blockwise tail ----
        with tc.tile_pool(name="dotp", bufs=2, space="PSUM") as dotp:
            nfb = nbb = 0
            for s in SLAB_ORDER:
                slab = dtp.tile([128, 16, 512], BF16, tag="dt")
                nc.sync.dma_start(slab[:], dt_d.ap()[:, s, :, :])
                if s == SLAB_ORDER[0]:
                    # constants queue behind the first slab: the slab DMA
                    # paces everything, the consts only gate the dots MMs
                    nc.sync.dma_start(cm_t[:], cm_d.ap())
                bank = dotp.tile([128, 512], F32, tag="bank")
                for g in range(4):
                    for c in range(4):
                        nc.tensor.matmul(
                            bank[32 * c:32 * c + 32, :],
                            wtc(g), slab[:, 4 * g + c, :],
                            start=(g == 0), stop=(g == 3),
                            tile_position=(0, 32 * c))
                nc.scalar.activation(e_t[:, 512 * s:512 * s + 512], bank[:],
                                     Act.Exp)
                nc.scalar.activation(einv_t[:, 512 * s:512 * s + 512],
                                     bank[:], Act.Exp, scale=-1.0)
                if s == 0:
                    nc.vector.tensor_copy(ae_t[:, blk(0)], e_t[:, blk(0)])
                if s == 7:
                    nc.vector.tensor_copy(be_t[:, blk(63)], e_t[:, blk(63)])
                # pump against the PREVIOUS slab's coverage (one-slab lag):
                # scan steps for bank k are emitted after slab k+1's dots
                # matmuls, so the serial scan chain never sits ahead of the
                # next slab's dots in the PE FIFO (head-of-line blocking).
                pump()
                if s == nfb:
                    nfb += 1
                if s == 7 - nbb:
                    nbb += 1
                st["fbanks"], st["bbanks"] = nfb, nbb

            # dn load: after all dt slabs in the single DMA FIFO, so the
            # scan-critical dt stream gets full bandwidth; dn lands during
            # the scan tail, middle-out to match dw block order.
            for q in BLOCK_ORDER:
                nc.sync.dma_start(dn_t[:, 16 * q:16 * q + 16, :],
                                  dn_d.ap()[:, 16 * q:16 * q + 16, :])

            st["fbanks"] = 8
            st["bbanks"] = 8
            pump()
            assert st["fwd"] > 63 and st["bwd"] < 0 and not st["blocks"], \
                f"emission incomplete: {st}"

        dw_sb = tgp.tile([128, 512], F32, tag="dwout")
        nc.scalar.activation(dw_sb[:], dw_ps[:], Act.Copy)
        nc.sync.dma_start(dw_d.ap(), dw_sb[:])


def kernel(W, T, data, labels):
    import ml_dtypes
    bf16 = ml_dtypes.bfloat16
    f8 = ml_dtypes.float8_e4m3

    W = np.asarray(W, np.float32)
    T = np.asarray(T, np.float32)
    data = np.asarray(data, np.float32)
    labels = np.asarray(labels, np.int64)

    ETs = np.exp(T.astype(np.float64)) / CHAT
    cm = np.zeros((128, 644), np.float32)
    for g in range(4):
        cm[:, 32 * g:32 * g + 32] = W.T[128 * g:128 * g + 128, :]   # wt
    for c in range(4):
        sl = slice(32 * c, 32 * c + 32)
        cm[sl, 128 + 32 * c:128 + 32 * c + 32] = ETs                # etf
        cm[sl, 256 + 32 * c:256 + 32 * c + 32] = ETs.T              # etb
        cm[sl, 512 + c] = 1.0                                       # oz
        cm[c, 516 + 32 * c:516 + 32 * c + 32] = 1.0                 # ob
    cm[:, 384:512] = np.eye(128, dtype=np.float32)                  # id128

    nc = _CACHE.get("nc")
    if nc is None:
        nc = _build_module()
        _CACHE["nc"] = nc

    in_maps = []
    for core in range(NC):
        dcore = data[core * WPC:(core + 1) * WPC]          # [256, 64, 512]
        # position-major permuted rows: (c, f=64i+w) <-> word 64c+w, pos i
        dn_perm = dcore.reshape(4, 64, 64, 512).transpose(0, 2, 1, 3)
        dn_perm = np.ascontiguousarray(dn_perm).reshape(4, 4096, 512)
        # dt [128, 8s, 16(4g+c), 512]: [p,s,g,c,fo] = dn_perm[c, 512s+fo, 128g+p]
        dt_arr = dn_perm.reshape(4, 8, 512, 4, 128).transpose(4, 1, 3, 0, 2)
        dt_arr = np.ascontiguousarray(dt_arr).reshape(128, 8, 16, 512)
        # dn [128, 128(t=4jj+c), 512]: [p, jj, c, d] = dn_perm[c, 128jj+p, d]
        dn_arr = dn_perm.reshape(4, 32, 128, 512).transpose(2, 1, 0, 3)
        dn_arr = np.ascontiguousarray(dn_arr).reshape(128, 128, 512)
        in_maps.append({
            "dt": dt_arr.astype(bf16),
            "dn": dn_arr.astype(f8),
            "cm": cm.astype(bf16),
        })

    _CACHE["last_in_maps"] = in_maps
    res = run_bass_kernel_spmd(nc, in_maps, list(range(NC)))
    results = res.results

    dwn_sum = np.zeros((K, D), np.float64)   # sum of p1.T @ x
    Mmat = np.zeros((K, K), np.float64)
    for core in range(NC):
        r = results[core]
        dwn_sum += r["dw"].astype(np.float64).reshape(4, K, D).sum(axis=0)
        ae = r["ae"].astype(np.float64).reshape(4, K, 64, 64)  # [c,k,i,w]
        be = r["be"].astype(np.float64).reshape(4, K, 64, 64)
        z = ae[:, :, 63, :].sum(axis=1)                        # [c, w]
        rz = 1.0 / z
        aer = ae[:, :, :63, :] * rz[:, None, None, :]
        ben = be[:, :, 1:, :]
        Mmat += np.einsum('ckiw,cliw->kl', aer, ben)

    # onehot.T @ data in full precision on the host (BLAS sgemm)
    lab_flat = labels.ravel()
    oh_mat = (lab_flat[:, None] == np.arange(K)[None, :]).astype(np.float32)
    dwoh = (oh_mat.T @ data.reshape(-1, D)).astype(np.float64)

    counts = np.bincount(
        (labels[:, :-1].ravel() * K + labels[:, 1:].ravel()).astype(np.int64),
        minlength=K * K).reshape(K, K).astype(np.float64)

    meandw = ((dwoh - dwn_sum) / N).astype(np.float32)
    meandT = ((counts - ETs * Mmat) / N).astype(np.float32)
    return np.concatenate([meandw.ravel(), meandT.ravel()]).astype(np.float32)


# revision 30
# speedup vs baseline: 1.1628x; 1.1628x over previous
"""CRF layer gradient kernel for 8 TRN2 NeuronCores (v3).

Strategy: data-parallel over the N=2048 words axis (256 words/core, as
4 chains x 64 words packed into the 128 partitions = 4 chains x 32 labels).
Free-dim packing is position-major within a chain: column f = 64*i + w
(i = position, w = word), so every scan step reads/writes one contiguous
[128, 64] slice.

The forward-backward DP runs in the exp domain (scaled by CHAT): with
ETs = exp(T)/CHAT,
  AE[0] = E[0];   AE[i] = (AE[i-1] @ ETs)  * E[i]
  BE[63] = E[63]; BE[i] = (BE[i+1] @ ETs.T) * E[i]      (unnormalized)
  p1[i] = AE[i] * BE[i] * exp(-dots[i]) * (1/z)
Forward and backward scans are independent and chase the DMA stream from
opposite ends (dt slabs arrive 0,7,1,6,...); z is computed at the meeting
point (z = sum_k AE[t]*BE[t]*Einv[t] at any t — exact identity), so the
normalizer is ready as soon as the scans cross.  The p1 -> transpose ->
dw pipeline then runs blockwise, middle-out, overlapping the scan tails:
DVE does the p1 products while PE does transposes + dw matmuls.

PE efficiency: the K=32-wide matmuls (emission scores and dw = p1.T @ x)
are 4-way column-tiled via tile_position=(0,32c), running the four chains
concurrently in separate 32-column groups of the 128x128 array.  p1 is
transposed with 32 full-width [128,128] PE transposes.  dn and p1 travel
as fp8e4; everything else bf16 (validated ~8e-3 vs the fp32 reference).

Device returns dwn = p1.T @ x (per-chain bands) and the AE/BE marginal
factors; host computes onehot.T @ x in full precision (BLAS), subtracts,
and folds the tiny dT matrix in float64.
"""

import sys

import numpy as np

sys.path.insert(0, "/opt/trn_rl_repo")

import concourse.bass as bass
import concourse.tile as tile
from concourse import bacc, mybir
from concourse.bass_utils import run_bass_kernel_spmd

N, M, K, D = 2048, 64, 32, 512
NC = 8
WPC = N // NC          # 256 words per core
RPC = WPC * M          # 16384 rows per core
CHAT = 60.0
F32 = mybir.dt.float32
BF16 = mybir.dt.bfloat16
F8 = mybir.dt.float8e4

_CACHE = {}

# dots-block arrival order: alternate ends so fwd (blocks 0,1,..) and bwd
# (blocks 7,6,..) both chase the stream; block 4 lands last.
SLAB_ORDER = [0, 7, 1, 6, 2, 5, 3, 4]
# p1/transpose/dw block order: middle-out, matching scan completion.
BLOCK_ORDER = [3, 4, 2, 5, 1, 6, 0, 7]


def _build_module():
    nc = bacc.Bacc("TRN2", target_bir_lowering=False, debug=False)

    dt_d = nc.dram_tensor("dt", [128, 8, 16, 512], BF16, kind="ExternalInput")
    dn_d = nc.dram_tensor("dn", [128, 128, 512], F8, kind="ExternalInput")
    # merged constants, one DMA: wt(128) | etf(128) | etb(128) | id(128)
    # | oz(4) | ob(128, rows 0-3)
    cm_d = nc.dram_tensor("cm", [128, 644], BF16, kind="ExternalInput")
    dw_d = nc.dram_tensor("dw", [128, 512], F32, kind="ExternalOutput")
    ae_d = nc.dram_tensor("ae", [128, 4096], BF16, kind="ExternalOutput")
    be_d = nc.dram_tensor("be", [128, 4096], BF16, kind="ExternalOutput")

    with tile.TileContext(nc) as tc:
        _kernel_body(tc, nc, dt_d, dn_d, cm_d, dw_d, ae_d, be_d)
    nc.compile()
    return nc


def _kernel_body(tc, nc, dt_d, dn_d, cm_d, dw_d, ae_d, be_d):
    from contextlib import ExitStack
    Act = mybir.ActivationFunctionType
    ctx = ExitStack()
    with ctx:
        consts = ctx.enter_context(tc.tile_pool(name="consts", bufs=1))
        big = ctx.enter_context(tc.tile_pool(name="big", bufs=1))
        dtp = ctx.enter_context(tc.tile_pool(name="dtp", bufs=3))

        cm_t = consts.tile([128, 644], BF16)

        def wtc(g):
            return cm_t[:, 32 * g:32 * g + 32]
        etf_ap = cm_t[:, 128:256]
        etb_ap = cm_t[:, 256:384]
        id_ap = cm_t[:, 384:512]
        oz_ap = cm_t[:, 512:516]
        ob_ap = cm_t[0:4, 516:644]

        e_t = big.tile([128, 4096], BF16, tag="e")
        einv_t = big.tile([128, 4096], BF16, tag="einv")
        ae_t = big.tile([128, 4096], BF16, tag="ae")
        be_t = big.tile([128, 4096], BF16, tag="be")
        p1_t = big.tile([128, 4096], BF16, tag="p1")
        dn_t = big.tile([128, 128, 512], F8, tag="dn")
        rzb_t = consts.tile([128, 64], BF16)

        def blk(t):            # contiguous 64-col slice for position t
            return slice(64 * t, 64 * t + 64)

        scn = ctx.enter_context(tc.tile_pool(name="scn", bufs=3, space="PSUM"))
        tgp = ctx.enter_context(tc.tile_pool(name="tgp", bufs=5))

        st = {"fwd": 1, "bwd": 62, "fbanks": 0, "bbanks": 0,
              "z_done": False, "blocks": list(BLOCK_ORDER), "nj": 0,
              "exported": False, "allow_blocks": False,
              "trp": None, "dw_ps": None}

        def emit_fwd(t):
            aps = scn.tile([128, 64], F32, tag="s")
            nc.tensor.matmul(aps[:], etf_ap, ae_t[:, blk(t - 1)],
                             start=True, stop=True)
            nc.vector.tensor_mul(ae_t[:, blk(t)], aps[:], e_t[:, blk(t)])

        def emit_bwd(u):
            bps = scn.tile([128, 64], F32, tag="s")
            nc.tensor.matmul(bps[:], etb_ap, be_t[:, blk(u + 1)],
                             start=True, stop=True)
            nc.vector.tensor_mul(be_t[:, blk(u)], bps[:], e_t[:, blk(u)])

        def emit_z():
            # z = sum_k AE[32]*BE[32]*Einv[32] (exact at any position);
            # broadcast 1/z to all partitions via ones-block matmuls.
            tmp = consts.tile([128, 64], BF16)
            nc.vector.tensor_mul(tmp[:], ae_t[:, blk(32)], be_t[:, blk(32)])
            nc.vector.tensor_mul(tmp[:], tmp[:], einv_t[:, blk(32)])
            z_ps = scn.tile([4, 64], F32, tag="s")
            nc.tensor.matmul(z_ps[:], oz_ap, tmp[:], start=True, stop=True)
            rz_s = consts.tile([4, 64], BF16)
            with nc.allow_low_precision(reason="rz bf16 validated to 8e-3"):
                nc.vector.reciprocal(rz_s[:], z_ps[:])
            rzb_ps = scn.tile([128, 64], F32, tag="s")
            nc.tensor.matmul(rzb_ps[:], ob_ap, rz_s[:], start=True,
                             stop=True)
            nc.vector.tensor_copy(rzb_t[:], rzb_ps[:])

        def emit_block(b):
            # p1 = (AE*BE) * (Einv*rz) on this 512-col block; AE*BE runs on
            # the otherwise-idle GpSimd engine, Einv*rz and the combine on
            # DVE.  Then transpose the four 128-col strips back-to-back
            # (keeps PE dense; tg copies overlap on ACT) and feed the
            # column-tiled dw matmul.
            L = slice(512 * b, 512 * b + 512)
            nc.gpsimd.tensor_mul(p1_t[:, L], ae_t[:, L], be_t[:, L])
            e3 = einv_t[:, L].rearrange("p (i w) -> p i w", i=8)
            rb = rzb_t[:].unsqueeze(1).broadcast_to([128, 8, 64])
            nc.vector.tensor_mul(e3, e3, rb)
            nc.vector.tensor_mul(p1_t[:, L], p1_t[:, L], einv_t[:, L])
            tgs = []
            for jj in range(4):
                j = 4 * b + jj
                tr = st["trp"].tile([128, 128], BF16, tag="tr")
                nc.tensor.transpose(tr[:], p1_t[:, 128 * j:128 * j + 128],
                                    id_ap)
                tg = tgp.tile([128, 128], F8, tag="tg")
                nc.scalar.activation(tg[:], tr[:], Act.Copy)
                tgs.append(tg)
            for jj in range(4):
                j = 4 * b + jj
                for c in range(4):
                    nc.tensor.matmul(
                        st["dw_ps"][32 * c:32 * c + 32, :],
                        tgs[jj][:, 32 * c:32 * c + 32], dn_t[:, 4 * j + c, :],
                        start=(st["nj"] == 0), stop=(st["nj"] == 31),
                        tile_position=(0, 32 * c))
                st["nj"] += 1

        def pump():
            # emit everything whose inputs are covered: scan steps, then z
            # at the crossing, then completed p1/dw blocks (middle-out).
            while True:
                can_f = st["fwd"] <= 63 and (st["fwd"] // 8) < st["fbanks"]
                can_b = st["bwd"] >= 0 and (st["bwd"] // 8) >= 8 - st["bbanks"]
                if not (can_f or can_b):
                    break
                if can_f:
                    emit_fwd(st["fwd"])
                    st["fwd"] += 1
                if can_b:
                    emit_bwd(st["bwd"])
                    st["bwd"] -= 1
                if not st["z_done"] and st["fwd"] > 32 and st["bwd"] < 32:
                    emit_z()
                    st["z_done"] = True
                while st["blocks"] and st["allow_blocks"]:
                    b = st["blocks"][0]
                    if st["fwd"] > 8 * b + 7 and st["bwd"] < 8 * b \
                            and st["z_done"]:
                        emit_block(st["blocks"].pop(0))
                    else:
                        break
            if st["fwd"] > 63 and st["bwd"] < 0 and not st["exported"]:
                # marginal factors out as soon as the scans finish, on the
                # scalar DMA queue so the final dw export (sync queue)
                # doesn't wait behind these 2 MB.
                nc.scalar.dma_start(ae_d.ap(), ae_t[:])
                nc.scalar.dma_start(be_d.ap(), be_t[:])
                st["exported"] = True

        # ---- emission-score stream + scans + blockwise tail ----
        with tc.tile_pool(name="dotp", bufs=2, space="PSUM") as dotp:
            nfb = nbb = 0
            for s in SLAB_ORDER:
                slab = dtp.tile([128, 16, 512], BF16, tag="dt")
                nc.sync.dma_start(slab[:], dt_d.ap()[:, s, :, :])
                if s == SLAB_ORDER[0]:
                    # constants queue behind the first slab: the slab DMA
                    # paces everything, the consts only gate the dots MMs
                    nc.sync.dma_start(cm_t[:], cm_d.ap())
                bank = dotp.tile([128, 512], F32, tag="bank")
                for g in range(4):
                    for c in range(4):
                        nc.tensor.matmul(
                            bank[32 * c:32 * c + 32, :],
                            wtc(g), slab[:, 4 * g + c, :],
                            start=(g == 0), stop=(g == 3),
                            tile_position=(0, 32 * c))
                nc.scalar.activation(e_t[:, 512 * s:512 * s + 512], bank[:],
                                     Act.Exp)
                nc.scalar.activation(einv_t[:, 512 * s:512 * s + 512],
                                     bank[:], Act.Exp, scale=-1.0)
                if s == 0:
                    nc.vector.tensor_copy(ae_t[:, blk(0)], e_t[:, blk(0)])
                if s == 7:
                    nc.vector.tensor_copy(be_t[:, blk(63)], e_t[:, blk(63)])
                # pump against the PREVIOUS slab's coverage (one-slab lag):
                # scan steps for bank k are emitted after slab k+1's dots
                # matmuls, so the serial scan chain never sits ahead of the
                # next slab's dots in the PE FIFO (head-of-line blocking).
                pump()
                if s == nfb:
                    nfb += 1
                if s == 7 - nbb:
                    nbb += 1
                st["fbanks"], st["bbanks"] = nfb, nbb

        # dn load: after all dt slabs in the single DMA FIFO, so the
        # scan-critical dt stream gets full bandwidth; dn lands during
        # the scan tail, middle-out to match dw block order.
        for q in BLOCK_ORDER:
            nc.sync.dma_start(dn_t[:, 16 * q:16 * q + 16, :],
                              dn_d.ap()[:, 16 * q:16 * q + 16, :])

        # the dot-bank pool is closed now, freeing PSUM for a deep
        # transpose pool (blocks are only emitted from here on)
        with tc.tile_pool(name="trp", bufs=4, space="PSUM") as trp, \
             tc.tile_pool(name="dwp", bufs=1, space="PSUM") as dwp:
            st["trp"] = trp
            st["dw_ps"] = dwp.tile([128, 512], F32, tag="dwps",
                                   name="dw_ps")
            st["allow_blocks"] = True
            st["fbanks"] = 8
            st["bbanks"] = 8
            pump()
            assert st["fwd"] > 63 and st["bwd"] < 0 and not st["blocks"], \
                f"emission incomplete: {st}"

            dw_sb = tgp.tile([128, 512], F32, tag="dwout")
            nc.scalar.activation(dw_sb[:], st["dw_ps"][:], Act.Copy)
        nc.sync.dma_start(dw_d.ap(), dw_sb[:])


def kernel(W, T, data, labels):
    import ml_dtypes
    bf16 = ml_dtypes.bfloat16
    f8 = ml_dtypes.float8_e4m3

    W = np.asarray(W, np.float32)
    T = np.asarray(T, np.float32)
    data = np.asarray(data, np.float32)
    labels = np.asarray(labels, np.int64)

    ETs = np.exp(T.astype(np.float64)) / CHAT
    cm = np.zeros((128, 644), np.float32)
    for g in range(4):
        cm[:, 32 * g:32 * g + 32] = W.T[128 * g:128 * g + 128, :]   # wt
    for c in range(4):
        sl = slice(32 * c, 32 * c + 32)
        cm[sl, 128 + 32 * c:128 + 32 * c + 32] = ETs                # etf
        cm[sl, 256 + 32 * c:256 + 32 * c + 32] = ETs.T              # etb
        cm[sl, 512 + c] = 1.0                                       # oz
        cm[c, 516 + 32 * c:516 + 32 * c + 32] = 1.0                 # ob
    cm[:, 384:512] = np.eye(128, dtype=np.float32)                  # id128

    nc = _CACHE.get("nc")
    if nc is None:
        nc = _build_module()
        _CACHE["nc"] = nc

    in_maps = []
    for core in range(NC):
        dcore = data[core * WPC:(core + 1) * WPC]          # [256, 64, 512]
        # position-major permuted rows: (c, f=64i+w) <-> word 64c+w, pos i
        dn_perm = dcore.reshape(4, 64, 64, 512).transpose(0, 2, 1, 3)
        dn_perm = np.ascontiguousarray(dn_perm).reshape(4, 4096, 512)
        # dt [128, 8s, 16(4g+c), 512]: [p,s,g,c,fo] = dn_perm[c, 512s+fo, 128g+p]
        dt_arr = dn_perm.reshape(4, 8, 512, 4, 128).transpose(4, 1, 3, 0, 2)
        dt_arr = np.ascontiguousarray(dt_arr).reshape(128, 8, 16, 512)
        # dn [128, 128(t=4jj+c), 512]: [p, jj, c, d] = dn_perm[c, 128jj+p, d]
        dn_arr = dn_perm.reshape(4, 32, 128, 512).transpose(2, 1, 0, 3)
        dn_arr = np.ascontiguousarray(dn_arr).reshape(128, 128, 512)
        in_maps.append({
            "dt": dt_arr.astype(bf16),
            "dn": dn_arr.astype(f8),
            "cm": cm.astype(bf16),
        })

    _CACHE["last_in_maps"] = in_maps
    res = run_bass_kernel_spmd(nc, in_maps, list(range(NC)))
    results = res.results

    dwn_sum = np.zeros((K, D), np.float64)   # sum of p1.T @ x
    Mmat = np.zeros((K, K), np.float64)
    for core in range(NC):
        r = results[core]
        dwn_sum += r["dw"].astype(np.float64).reshape(4, K, D).sum(axis=0)
        ae = r["ae"].astype(np.float64).reshape(4, K, 64, 64)  # [c,k,i,w]
        be = r["be"].astype(np.float64).reshape(4, K, 64, 64)
        z = ae[:, :, 63, :].sum(axis=1)                        # [c, w]
        rz = 1.0 / z
        aer = ae[:, :, :63, :] * rz[:, None, None, :]
        ben = be[:, :, 1:, :]
        Mmat += np.einsum('ckiw,cliw->kl', aer, ben)

    # onehot.T @ data in full precision on the host (BLAS sgemm)
    lab_flat = labels.ravel()
    oh_mat = (lab_flat[:, None] == np.arange(K)[None, :]).astype(np.float32)
    dwoh = (oh_mat.T @ data.reshape(-1, D)).astype(np.float64)

    counts = np.bincount(
        (labels[:, :-1].ravel() * K + labels[:, 1:].ravel()).astype(np.int64),
        minlength=K * K).reshape(K, K).astype(np.float64)

    meandw = ((dwoh - dwn_sum) / N).astype(np.float32)
    meandT = ((counts - ETs * Mmat) / N).astype(np.float32)
    return np.concatenate([meandw.ravel(), meandT.ravel()]).astype(np.float32)
